# revision 18
# baseline (speedup 1.0000x reference)
"""Conv2d(128->256, 3x3, pad 1) with LoRA (rank 8) — Trainium2 Bass kernel.

Strategy:
  - Data-parallel over batch: 16 images -> 2 per core x 8 cores. Conv weights
    and LoRA A/B replicated.
  - LoRA folds into the conv weight (conv is linear in weights):
        W_eff = W + (alpha/rank) * (B @ A).reshape(C_OUT, C_IN, 3, 3)
    computed on-device with 9 tiny PE matmuls + fused DVE add.
  - The 3x3 conv itself = 9 shifted matmuls accumulating in PSUM:
        out[co, pix] += W_eff[co, :, kh, kw]^T @ x_shift[ci, pix]
    with K = C_IN = 128 (partition dim), M = 128 (co block), N = 512
    (8 image rows x 64 cols) in float32r (full PE rate at N >= 256).
  - Host-side prep is layout only (zero-padding + transposes), no arithmetic.
"""

import numpy as np

import concourse.bass as bass
import concourse.tile as tile
from concourse import bacc, mybir
from concourse.bass_utils import run_bass_kernel_spmd

N_CORES = 8
B, C_IN, H, W_DIM = 16, 128, 64, 64
C_OUT = 256
RANK = 8
SCALING = 2.0  # alpha/rank = 16/8
HP, WP = H + 2, W_DIM + 2  # zero-padded image dims
B_LOC = B // N_CORES  # images per core
NPIX = H * W_DIM  # 4096
ROWS_PER_TILE = 8  # output rows per matmul group -> N = 8*64 = 512
N_RG = H // ROWS_PER_TILE  # 8 row groups

F32 = mybir.dt.float32
F32R = mybir.dt.float32r
IDENT = mybir.ActivationFunctionType.Identity


def _build_nc():
    nc = bacc.Bacc(
        "TRN2",
        target_bir_lowering=False,
        debug=False,
        num_devices=N_CORES,
    )

    xp = nc.dram_tensor("xp", [B_LOC, C_IN, HP * WP], F32, kind="ExternalInput").ap()
    wt = nc.dram_tensor("wt", [C_IN, 9 * C_OUT], F32, kind="ExternalInput").ap()
    at = nc.dram_tensor("at", [RANK, 9 * C_IN], F32, kind="ExternalInput").ap()
    bt = nc.dram_tensor("bt", [RANK, C_OUT], F32, kind="ExternalInput").ap()
    bv = nc.dram_tensor("bv", [C_OUT], F32, kind="ExternalInput").ap()
    out = nc.dram_tensor("out", [B_LOC, C_OUT, NPIX], F32, kind="ExternalOutput").ap()

    with tile.TileContext(nc) as tc:
        with (
            tc.tile_pool(name="persist", bufs=1) as persist,
            tc.tile_pool(name="outp", bufs=4) as outp,
            tc.tile_pool(name="psum", bufs=7, space="PSUM") as psum,
        ):
            # --- persistent SBUF tiles -------------------------------------
            # f32r operands must be produced by a rounding compute op (BIR
            # verifier rule), so x is staged f32 then DVE-converted to f32r.
            x_sb = [
                persist.tile([C_IN, HP * WP], F32, name=f"x_sb{i}")
                for i in range(B_LOC)
            ]
            x_sbr = [
                persist.tile([C_IN, HP * WP], F32R, name=f"x_sbr{i}")
                for i in range(B_LOC)
            ]
            wt_sb = persist.tile([C_IN, 9 * C_OUT], F32, name="wt_sb")
            weff = persist.tile([C_IN, 9 * C_OUT], F32R, name="weff")
            at_sb = persist.tile([RANK, 9 * C_IN], F32, name="at_sb")
            bt_sb = persist.tile([RANK, C_OUT], F32, name="bt_sb")
            b_sb = persist.tile([128, 2], F32, name="b_sb")

            # --- PE warm-up ------------------------------------------------
            # The HAM clock gate holds the PE at 1.2 GHz until it has been
            # busy ~3.4us. Dummy matmuls on a zeroed scratch tile have no DMA
            # dependencies, so they warm the PE during the input prefetch.
            warm_sb = persist.tile([128, 512], F32, name="warm_sb")
            nc.gpsimd.memset(warm_sb[:], 0.0)
            warm_ps = psum.tile([128, 512], F32, tag="warm", bufs=1, name="warm_ps")
            for _ in range(4):
                nc.tensor.matmul(
                    warm_ps[:], warm_sb[:, :128], warm_sb[:], start=True, stop=True
                )

            # --- input DMAs ------------------------------------------------
            # Three DMA paths run in parallel:
            #   SWDGE q0 (gpsimd): weights + LoRA operands (off the HW queues)
            #   SWDGE q1-q3: image 1 (needed ~35us in)
            #   HWDGE sync+scalar: image 0 chunks, then all output tiles
            qs = [nc.sync, nc.scalar]
            nc.gpsimd.dma_start(at_sb[:], at)
            nc.gpsimd.dma_start(bt_sb[:], bt)
            for cb in range(2):
                nc.gpsimd.dma_start(
                    b_sb[:, cb : cb + 1],
                    bv[cb * 128 : (cb + 1) * 128].unsqueeze(1),
                )
            nc.gpsimd.dma_start(wt_sb[:], wt)

            N_CHUNK = 4
            csz = (HP * WP + N_CHUNK - 1) // N_CHUNK
            chunks = [
                (i, c * csz, min((c + 1) * csz, HP * WP))
                for i in range(B_LOC)
                for c in range(N_CHUNK)
            ]
            for i, lo, hi in chunks:
                c = lo // csz
                if i == 0:
                    qs[c % 2].dma_start(x_sb[i][:, lo:hi], xp[i, :, lo:hi])
                else:
                    nc.gpsimd.dma_start(x_sb[i][:, lo:hi], xp[i, :, lo:hi])

            # --- fold LoRA into the conv weight ----------------------------
            # weff[:, k*256+co] = wt[:, k*256+co] + 2 * (A_k^T @ B^T)[ci, co]
            # (plain fp32 matmuls: tiny, and they extend the PE warm-up)
            for k in range(9):
                lps = psum.tile([128, C_OUT], F32, tag="ps", name=f"lps{k}")
                nc.tensor.matmul(
                    lps[:],
                    at_sb[:, k * 128 : (k + 1) * 128],
                    bt_sb[:],
                    start=True,
                    stop=True,
                )
                nc.vector.scalar_tensor_tensor(
                    weff[:, k * C_OUT : (k + 1) * C_OUT],
                    lps[:],
                    SCALING,
                    wt_sb[:, k * C_OUT : (k + 1) * C_OUT],
                    op0=mybir.AluOpType.mult,
                    op1=mybir.AluOpType.add,
                )

            # x chunk conversions f32 -> f32r (rounding producer for the PE)
            for i, lo, hi in chunks:
                nc.vector.tensor_copy(x_sbr[i][:, lo:hi], x_sb[i][:, lo:hi])

            # --- the conv: 9 accumulating shift-matmuls per output tile ----
            for img in range(B_LOC):
                x_r = x_sbr[img][:].rearrange("p (h w) -> p h w", w=WP)
                for cb in range(2):
                    for rg in range(N_RG):
                        ps = psum.tile([128, 512], F32, tag="ps", name=f"ps{img}_{cb}_{rg}")
                        h0 = rg * ROWS_PER_TILE
                        for k in range(9):
                            dh, dw = k // 3 - 1, k % 3 - 1
                            rhs = x_r[
                                :,
                                h0 + 1 + dh : h0 + 1 + dh + ROWS_PER_TILE,
                                1 + dw : 65 + dw,
                            ]
                            lhsT = weff[:, k * 256 + cb * 128 : k * 256 + cb * 128 + 128]
                            nc.tensor.matmul(
                                ps[:],
                                lhsT,
                                rhs,
                                start=(k == 0),
                                stop=(k == 8),
                            )
                        o = outp.tile([128, 512], F32, tag="o", name=f"o{img}_{cb}_{rg}")
                        ti = (img * 2 + cb) * N_RG + rg
                        # Alternate the PSUM->SBUF bias-add between ACT and DVE
                        # so neither engine limits the drain of PSUM banks.
                        if ti % 2 == 0:
                            nc.scalar.activation(
                                o[:], ps[:], IDENT, bias=b_sb[:, cb : cb + 1]
                            )
                        else:
                            nc.vector.tensor_scalar_add(
                                o[:], ps[:], b_sb[:, cb : cb + 1]
                            )
                        qs[ti % 2].dma_start(
                            out[img, cb * 128 : (cb + 1) * 128, rg * 512 : (rg + 1) * 512],
                            o[:],
                        )

    nc.compile()
    return nc


_NC_CACHE = None


def _get_nc():
    global _NC_CACHE
    if _NC_CACHE is None:
        _NC_CACHE = _build_nc()
    return _NC_CACHE


def _host_prep(x, W, b, lora_A, lora_B):
    """Layout-only host prep (pad + transpose); no arithmetic."""
    x = np.ascontiguousarray(x, dtype=np.float32)
    xp_all = np.zeros((B, C_IN, HP, WP), dtype=np.float32)
    xp_all[:, :, 1 : H + 1, 1 : W_DIM + 1] = x
    xp_all = xp_all.reshape(B, C_IN, HP * WP)

    # [co, ci, kh, kw] -> [ci, k, co]
    wt = np.ascontiguousarray(
        np.asarray(W, dtype=np.float32).reshape(C_OUT, C_IN, 9).transpose(1, 2, 0)
    ).reshape(C_IN, 9 * C_OUT)
    # [r, ci*9+k] -> [r, k, ci]
    at = np.ascontiguousarray(
        np.asarray(lora_A, dtype=np.float32).reshape(RANK, C_IN, 9).transpose(0, 2, 1)
    ).reshape(RANK, 9 * C_IN)
    # [co, r] -> [r, co]
    bt = np.ascontiguousarray(np.asarray(lora_B, dtype=np.float32).T)
    bv = np.ascontiguousarray(np.asarray(b, dtype=np.float32))
    return xp_all, wt, at, bt, bv


def run(x, W, b, lora_A, lora_B, trace=False):
    """Run the kernel on 8 cores; returns (full_output, BassKernelResults)."""
    xp_all, wt, at, bt, bv = _host_prep(x, W, b, lora_A, lora_B)
    nc = _get_nc()
    in_maps = []
    for c in range(N_CORES):
        in_maps.append(
            {
                "xp": np.ascontiguousarray(xp_all[c * B_LOC : (c + 1) * B_LOC]),
                "wt": wt,
                "at": at,
                "bt": bt,
                "bv": bv,
            }
        )
    res = run_bass_kernel_spmd(
        nc, in_maps, core_ids=list(range(N_CORES)), trace=trace
    )
    out = np.concatenate([r["out"] for r in res.results], axis=0)
    return out.reshape(B, C_OUT, H, W_DIM), res


def kernel(x, W, b, lora_A, lora_B):
    out, _ = run(x, W, b, lora_A, lora_B, trace=False)
    return out
